# revision 19
# baseline (speedup 1.0000x reference)
"""AnchorLoss distributed Trainium2 kernel (8 NeuronCores).

reference math (anchors: [8192, 8, 512] f32):
    x = anchors.reshape(8192, 4096)
    loss = -(2*N*sum(x*x) - 2*sum(colsum(x)^2)) / sqrt(512)

Strategy: shard COLUMNS across the 8 cores (512 columns each, slice
[8192, 512] f32 = 16 MiB per core). Cores never communicate; each core
ships a per-partition partial pack [128, 136] f32 to DRAM and the host
finishes the (tiny) reduction in fp64.

Per-core pipeline:
  1. Pool SWDGE casts the whole f32 slice to fp8e4 DRAM->DRAM in ONE
     DMA into a row-padded scratch ([8192, 516] fp8, cols 0:512; the
     pad keeps the access pattern un-mergeable so the 512B-row shape
     survives lowering and the transfer is charged at the descriptor
     minimum).  HBM reads all 16 MiB once; fp8 quantization noise is
     ~0.1% on sum(x^2), far inside the 2e-2 gate.
  2. Rows 0:512 are loaded as two f32 half-tiles on the SP / Act HWDGE
     queues (filling them until the cast lands) and squared on DVE.
     Rows 512:8192 stream as fp8 tiles [128, 4, 512]: 6 on Pool, 4.5
     on SP, 4.5 on Act (the .5 are [128, 2, 512] tail halves), so all
     three queues drain at the same time.
  3. PE consumes every fp8 piece with DoubleRow gram matmuls
     (lhsT = rhs = [p, 2, 128-chunk]; the PSUM diagonal accumulates
     sum(x^2), 27 ns each once the p-state ramp is warm) plus
     one-column matmuls against a ones vector for the per-column sums.
     Gram diag and colsums share one PSUM bank [128, 132].
  4. Tail: one DVE copy moves the bank into the pack tile; one SP DMA
     writes pack [128, 136] to DRAM "out".

Host: loss = -(2*N*sumsq - 2*sum(colsum^2))/sqrt(512) in fp64 from the
8 packs (sumsq = gram diagonals + f32 row partials).
"""

import numpy as np

from concourse import bacc, bass, tile, mybir
from concourse.bass_utils import run_bass_kernel_spmd

# The axon client container has no /dev/neuron*, so the driver ioctls
# behind these routing lookups fail. The simulator only needs a sane
# single-device identity mapping (8 cores on device 0); the real NEFF
# resolves routing on-device and never reads these.
import concourse.libnrt as _lnrt
import concourse.bass_interp as _bi
try:
    _lnrt.get_trn2_nc_mapping()
except Exception:
    _IDENT = {(0, i): i for i in range(8)}
    _RID = {0: 0}
    _lnrt.get_trn2_nc_mapping = lambda: _IDENT
    _lnrt.get_device_id_to_routing_id_mapping = lambda: _RID
    _bi.get_device_id_to_routing_id_mapping = lambda: _RID

N_CORES = 8
N_CLASSES = 8192
D = 4096                          # 8 * 512 flattened embedding dim
COLS = D // N_CORES               # 512 columns per core
P = 128                           # partitions
RB = 4                            # row-blocks per full fp8 tile
TILE_ROWS = P * RB                # 512 rows per full tile
HEAD_ROWS = 512                   # rows 0:512 go f32 on SP/Act
CHUNK = 128
N_CHUNKS = COLS // CHUNK          # 4
FACTOR = float(np.sqrt(np.float64(512.0)))
PACK_W = 136                      # 128 gram + 4 colsum + 2 rowsumsq (+2 pad)
BANK_W = CHUNK + N_CHUNKS         # shared PSUM bank: gram diag + colsums

N_WARM = 16                       # PE p-state warmup matmuls

# fp8 pieces: (queue, start_row, n_row_blocks). 6 full tiles on Pool,
# 4 full + 1 half on each of SP/Act; halves last for tail shaping.
_R = HEAD_ROWS


def _fp8_schedule():
    # interleave by estimated arrival: Pool starts right after the cast
    # (~600), SP/Act after their f32 head piece (~1779).
    sched = []
    r = _R
    qstate = {"gpsimd": [600.0, 6 * RB], "sync": [1779.0, 4 * RB + 2],
              "scalar": [1779.0, 4 * RB + 2]}
    while any(v[1] > 0 for v in qstate.values()):
        # pick queue that finishes its next piece earliest
        best = None
        for eng, (t, left) in qstate.items():
            if left <= 0:
                continue
            nrb = min(RB, left)
            # don't leave a lone 2-block tail on pool (it has multiples of 4)
            dur = 790.0 * nrb / RB
            dur = max(dur, 500.0)
            cand = (t + dur, eng, nrb)
            if best is None or cand < best:
                best = cand
        t_end, eng, nrb = best
        qstate[eng][0] = t_end
        qstate[eng][1] -= nrb
        sched.append((t_end, eng, r, nrb))
        r += nrb * P
    assert r == N_CLASSES, r
    return sched


def _build():
    nc = bacc.Bacc(None, num_devices=N_CORES)
    x_ext = nc.declare_dram_parameter(
        "anchors", [N_CLASSES, COLS], mybir.dt.float32, isOutput=False
    )
    out_ext = nc.declare_dram_parameter(
        "out", [P, PACK_W], mybir.dt.float32, isOutput=True
    )
    # row-padded fp8 scratch: stride 516 keeps rows non-contiguous so
    # balance_dma_aps leaves the cast AP at [[516, 8192], [1, 512]]
    scr8 = nc.dram_tensor("scr8", (N_CLASSES, COLS + 4), mybir.dt.float8e4,
                          kind="Internal")

    with tile.TileContext(nc) as tc:
        with (
            tc.tile_pool(name="io", bufs=1) as io,
            tc.tile_pool(name="psum", bufs=1, space="PSUM") as ps,
        ):
            # ---- constants / setup (DVE) ----
            wones = io.tile([P, CHUNK], mybir.dt.bfloat16)
            nc.vector.memset(wones[:], 0.001)
            ones8 = io.tile([P, 1], mybir.dt.float8e4)
            nc.vector.memset(ones8[:], 1.0)
            ones_f = io.tile([P, 1], mybir.dt.float32)
            nc.vector.memset(ones_f[:], 1.0)
            pack = io.tile([P, PACK_W], mybir.dt.float32)
            nc.vector.memset(pack[:], 0.0)

            # ---- the one-shot DRAM->DRAM fp8 cast (Pool queue) ----
            nc.gpsimd.dma_start(scr8[:, 0:COLS], x_ext[:, :])

            # ---- PE p-state warmup ----
            warm_ps = ps.tile([P, CHUNK], mybir.dt.float32)
            for _ in range(N_WARM):
                nc.tensor.matmul(warm_ps[:], lhsT=wones[:], rhs=wones[:],
                                 start=True, stop=True)

            # one PSUM tile, one accumulation group: [:, 0:128] gram diag,
            # [:, 128:132] per-chunk colsums.  start only on the globally
            # first matmul (zeroes the whole tile region), stop on the last.
            bankGC = ps.tile([P, BANK_W], mybir.dt.float32, name="bankGC")

            # ---- head: rows 0:512 as two f32 half-tiles ----
            h0 = io.tile([P, 2, COLS], mybir.dt.float32)
            nc.sync.dma_start(
                h0[:], x_ext[0:256, :].rearrange("(rb p) c -> p rb c",
                                                 rb=2, p=P))
            h1 = io.tile([P, 2, COLS], mybir.dt.float32)
            nc.scalar.dma_start(
                h1[:], x_ext[256:512, :].rearrange("(rb p) c -> p rb c",
                                                   rb=2, p=P))

            sched = _fp8_schedule()
            n_mm = 8 * 2 + sum(6 * nrb for _, _, _, nrb in sched)

            # colsum of the f32 heads first (they land earliest)
            mm = 0
            for h in (h0, h1):
                for c in range(N_CHUNKS):
                    for j in range(2):
                        mm += 1
                        nc.tensor.matmul(
                            bankGC[:, CHUNK + c:CHUNK + c + 1],
                            lhsT=h[:, j, c * CHUNK:(c + 1) * CHUNK],
                            rhs=ones_f[:],
                            start=(mm == 1), stop=(mm == n_mm))

            for _, eng, r0, nrb in sched:
                xt = io.tile([P, nrb, COLS], mybir.dt.float8e4,
                             name=f"x8_{r0}")
                src = scr8[r0:r0 + nrb * P, 0:COLS].rearrange(
                    "(rb p) c -> p rb c", rb=nrb, p=P)
                getattr(nc, eng).dma_start(xt[:], src)
                for c in range(N_CHUNKS):
                    sl = slice(c * CHUNK, (c + 1) * CHUNK)
                    for jp in range(0, nrb, 2):
                        mm += 1
                        nc.tensor.matmul(
                            bankGC[:, 0:CHUNK],
                            lhsT=xt[:, jp:jp + 2, sl],
                            rhs=xt[:, jp:jp + 2, sl],
                            perf_mode=mybir.MatmulPerfMode.DoubleRow,
                            start=(mm == 1), stop=(mm == n_mm))
                    for j in range(nrb):
                        mm += 1
                        nc.tensor.matmul(
                            bankGC[:, CHUNK + c:CHUNK + c + 1],
                            lhsT=xt[:, j, sl],
                            rhs=ones8[:],
                            start=(mm == 1), stop=(mm == n_mm))

            # ---- head squares on DVE (one op each, accum into pack) ----
            sq0 = io.tile([P, 2, COLS], mybir.dt.bfloat16)
            nc.vector.scalar_tensor_tensor(
                out=sq0[:], in0=h0[:], scalar=1.0, in1=h0[:],
                op0=mybir.AluOpType.mult, op1=mybir.AluOpType.mult,
                accum_out=pack[:, 132:133])
            sq1 = io.tile([P, 2, COLS], mybir.dt.bfloat16)
            nc.vector.scalar_tensor_tensor(
                out=sq1[:], in0=h1[:], scalar=1.0, in1=h1[:],
                op0=mybir.AluOpType.mult, op1=mybir.AluOpType.mult,
                accum_out=pack[:, 133:134])

            # ---- tail: one PSUM -> pack copy, one out DMA ----
            nc.vector.tensor_copy(pack[:, 0:BANK_W], bankGC[:])
            nc.sync.dma_start(out_ext[:, :], pack[:])

    nc.finalize()
    return nc


_NC_CACHE = None


def _get_nc():
    global _NC_CACHE
    if _NC_CACHE is None:
        _NC_CACHE = _build()
    return _NC_CACHE


def _finish(packs):
    """Host-side fp64 reduction of the 8 per-core packs."""
    sumsq = 0.0
    colsum_sq = 0.0
    for k in range(N_CORES):
        pk = np.asarray(packs[k], dtype=np.float64)
        g = pk[:, 0:CHUNK]
        sumsq += float(np.trace(g))
        sumsq += float(pk[:, 132:134].sum())
        c = pk[:, CHUNK:CHUNK + N_CHUNKS]          # [q, chunk]
        colsum_sq += float((c * c).sum())
    total = 2.0 * N_CLASSES * sumsq - 2.0 * colsum_sq
    return np.float32(-total / FACTOR)


_LAST_RETRIES = 0


def _run(anchors: np.ndarray, trace: bool = False):
    """Returns (loss_scalar, BassKernelResults)."""
    global _LAST_RETRIES
    x = np.asarray(anchors, dtype=np.float32).reshape(N_CLASSES, D)
    in_maps = [
        {"anchors": np.ascontiguousarray(x[:, i * COLS:(i + 1) * COLS])}
        for i in range(N_CORES)
    ]
    nc = _get_nc()
    # The very first NEFF execution on a freshly-initialized device has
    # been seen to return garbage packs (uninitialized-memory flake).
    # Re-run until the packs pass integrity checks (finite, and the
    # sum-of-squares partials non-negative); steady-state runs are clean.
    _LAST_RETRIES = 0

    def _ok(p):
        if not np.isfinite(p).all():
            return False
        d = np.diagonal(p[:, 0:CHUNK])
        return d.min() > -1e-3 and p[:, 132:134].min() > -1e-3

    for attempt in range(4):
        res = run_bass_kernel_spmd(nc, in_maps,
                                   core_ids=list(range(N_CORES)),
                                   trace=trace)
        packs = [r["out"] for r in res.results]
        if all(_ok(np.asarray(p)) for p in packs):
            break
        _LAST_RETRIES = attempt + 1
    loss = _finish(packs)
    return loss, res


def kernel(anchors: np.ndarray) -> np.ndarray:
    loss, _ = _run(anchors)
    return np.asarray(loss, dtype=np.float32).reshape(())


# revision 28
# speedup vs baseline: 1.0073x; 1.0073x over previous
"""AnchorLoss distributed Trainium2 kernel (8 NeuronCores).

reference math (anchors: [8192, 8, 512] f32):
    x = anchors.reshape(8192, 4096)
    loss = -(2*N*sum(x*x) - 2*sum(colsum(x)^2)) / sqrt(512)

Strategy: shard COLUMNS across the 8 cores (512 columns each, slice
[8192, 512] f32 = 16 MiB per core). Cores never communicate; each core
ships a per-partition partial pack [128, 136] f32 to DRAM and the host
finishes the (tiny) reduction in fp64.

Per-core pipeline:
  1. Pool SWDGE casts the whole f32 slice to fp8e4 DRAM->DRAM in ONE
     DMA into a row-padded scratch ([8192, 516] fp8, cols 0:512; the
     pad keeps the access pattern un-mergeable so the 512B-row shape
     survives lowering and the transfer is charged at the descriptor
     minimum).  HBM reads all 16 MiB once; fp8 quantization noise is
     ~0.1% on sum(x^2), far inside the 2e-2 gate.
  2. Rows 0:512 are loaded as two f32 half-tiles on the SP / Act HWDGE
     queues (filling them until the cast lands) and squared on DVE.
     Rows 512:8192 stream as fp8 tiles [128, 4, 512]: 6 on Pool, 4.5
     on SP, 4.5 on Act (the .5 are [128, 2, 512] tail halves), so all
     three queues drain at the same time.
  3. PE consumes every fp8 piece with DoubleRow gram matmuls
     (lhsT = rhs = [p, 2, 128-chunk]; the PSUM diagonal accumulates
     sum(x^2), 27 ns each once the p-state ramp is warm) plus
     one-column matmuls against a ones vector for the per-column sums.
     Gram diag and colsums share one PSUM bank [128, 132].
  4. Tail: one DVE copy moves the bank into the pack tile; one SP DMA
     writes pack [128, 136] to DRAM "out".

Host: loss = -(2*N*sumsq - 2*sum(colsum^2))/sqrt(512) in fp64 from the
8 packs (sumsq = gram diagonals + f32 row partials).
"""

import numpy as np

from concourse import bacc, bass, tile, mybir
from concourse.bass_utils import run_bass_kernel_spmd

# The axon client container has no /dev/neuron*, so the driver ioctls
# behind these routing lookups fail. The simulator only needs a sane
# single-device identity mapping (8 cores on device 0); the real NEFF
# resolves routing on-device and never reads these.
import concourse.libnrt as _lnrt
import concourse.bass_interp as _bi
try:
    _lnrt.get_trn2_nc_mapping()
except Exception:
    _IDENT = {(0, i): i for i in range(8)}
    _RID = {0: 0}
    _lnrt.get_trn2_nc_mapping = lambda: _IDENT
    _lnrt.get_device_id_to_routing_id_mapping = lambda: _RID
    _bi.get_device_id_to_routing_id_mapping = lambda: _RID

N_CORES = 8
N_CLASSES = 8192
D = 4096                          # 8 * 512 flattened embedding dim
COLS = D // N_CORES               # 512 columns per core
P = 128                           # partitions
RB = 4                            # row-blocks per full fp8 tile
TILE_ROWS = P * RB                # 512 rows per full tile
HEAD_ROWS = 512                   # rows 0:512 go f32 on SP/Act
CHUNK = 128
N_CHUNKS = COLS // CHUNK          # 4
FACTOR = float(np.sqrt(np.float64(512.0)))
PACK_W = 136                      # 128 gram + 4 colsum + 2 rowsumsq (+2 pad)
BANK_W = CHUNK + N_CHUNKS         # shared PSUM bank: gram diag + colsums

N_WARM = 30                       # PE p-state warmup matmuls
WARM_W = 64                       # warmup matmul width (tunes warmup end)

# fp8 pieces: (queue, start_row, n_row_blocks). 6 full tiles on Pool,
# 4 full + 1 half on each of SP/Act; halves last for tail shaping.
_R = HEAD_ROWS


def _fp8_schedule():
    # interleave by estimated arrival: Pool starts right after the cast
    # (~600), SP/Act after their f32 head piece (~1779).
    sched = []
    r = _R
    qstate = {"gpsimd": [600.0, 6 * RB], "sync": [1779.0, 4 * RB + 2],
              "scalar": [1779.0, 4 * RB + 2]}
    while any(v[1] > 0 for v in qstate.values()):
        # pick queue that finishes its next piece earliest
        best = None
        for eng, (t, left) in qstate.items():
            if left <= 0:
                continue
            nrb = min(RB, left)
            # don't leave a lone 2-block tail on pool (it has multiples of 4)
            dur = 790.0 * nrb / RB
            dur = max(dur, 500.0)
            cand = (t + dur, eng, nrb)
            if best is None or cand < best:
                best = cand
        t_end, eng, nrb = best
        qstate[eng][0] = t_end
        qstate[eng][1] -= nrb
        sched.append((t_end, eng, r, nrb))
        r += nrb * P
    assert r == N_CLASSES, r
    return sched


def _build():
    nc = bacc.Bacc(None, num_devices=N_CORES)
    x_ext = nc.declare_dram_parameter(
        "anchors", [N_CLASSES, COLS], mybir.dt.float32, isOutput=False
    )
    out_ext = nc.declare_dram_parameter(
        "out", [P, PACK_W], mybir.dt.float32, isOutput=True
    )
    # row-padded fp8 scratch: stride 516 keeps rows non-contiguous so
    # balance_dma_aps leaves the cast AP at [[516, 8192], [1, 512]]
    scr8 = nc.dram_tensor("scr8", (N_CLASSES, COLS + 4), mybir.dt.float8e4,
                          kind="Internal")

    with tile.TileContext(nc) as tc:
        with (
            tc.tile_pool(name="io", bufs=1) as io,
            tc.tile_pool(name="psum", bufs=1, space="PSUM") as ps,
        ):
            # ---- constants / setup (DVE) ----
            wones = io.tile([P, CHUNK], mybir.dt.bfloat16)
            nc.vector.memset(wones[:], 0.001)
            ones8 = io.tile([P, 1], mybir.dt.float8e4)
            nc.vector.memset(ones8[:], 1.0)
            ones_f = io.tile([P, 1], mybir.dt.float32)
            nc.vector.memset(ones_f[:], 1.0)
            pack = io.tile([P, PACK_W], mybir.dt.float32)
            nc.vector.memset(pack[:], 0.0)

            # ---- the one-shot DRAM->DRAM fp8 cast (Pool queue) ----
            nc.gpsimd.dma_start(scr8[:, 0:COLS], x_ext[:, :])

            # ---- PE p-state warmup ----
            warm_ps = ps.tile([P, CHUNK], mybir.dt.float32)
            for _ in range(N_WARM):
                nc.tensor.matmul(warm_ps[:, 0:WARM_W], lhsT=wones[:],
                                 rhs=wones[:, 0:WARM_W],
                                 start=True, stop=True)

            # one PSUM tile, one accumulation group: [:, 0:128] gram diag,
            # [:, 128:132] per-chunk colsums.  start only on the globally
            # first matmul (zeroes the whole tile region), stop on the last.
            bankGC = ps.tile([P, BANK_W], mybir.dt.float32, name="bankGC")

            # ---- head: rows 0:512 as two f32 half-tiles ----
            h0 = io.tile([P, 2, COLS], mybir.dt.float32)
            nc.sync.dma_start(
                h0[:], x_ext[0:256, :].rearrange("(rb p) c -> p rb c",
                                                 rb=2, p=P))
            h1 = io.tile([P, 2, COLS], mybir.dt.float32)
            nc.scalar.dma_start(
                h1[:], x_ext[256:512, :].rearrange("(rb p) c -> p rb c",
                                                   rb=2, p=P))

            sched = _fp8_schedule()
            n_mm = 8 * 2 + sum(6 * nrb for _, _, _, nrb in sched)

            # colsum of the f32 heads first (they land earliest)
            mm = 0
            for h in (h0, h1):
                for c in range(N_CHUNKS):
                    for j in range(2):
                        mm += 1
                        nc.tensor.matmul(
                            bankGC[:, CHUNK + c:CHUNK + c + 1],
                            lhsT=h[:, j, c * CHUNK:(c + 1) * CHUNK],
                            rhs=ones_f[:],
                            start=(mm == 1), stop=(mm == n_mm))

            for _, eng, r0, nrb in sched:
                xt = io.tile([P, nrb, COLS], mybir.dt.float8e4,
                             name=f"x8_{r0}")
                src = scr8[r0:r0 + nrb * P, 0:COLS].rearrange(
                    "(rb p) c -> p rb c", rb=nrb, p=P)
                getattr(nc, eng).dma_start(xt[:], src)
                for c in range(N_CHUNKS):
                    sl = slice(c * CHUNK, (c + 1) * CHUNK)
                    for jp in range(0, nrb, 2):
                        mm += 1
                        nc.tensor.matmul(
                            bankGC[:, 0:CHUNK],
                            lhsT=xt[:, jp:jp + 2, sl],
                            rhs=xt[:, jp:jp + 2, sl],
                            perf_mode=mybir.MatmulPerfMode.DoubleRow,
                            start=(mm == 1), stop=(mm == n_mm))
                    for j in range(nrb):
                        mm += 1
                        nc.tensor.matmul(
                            bankGC[:, CHUNK + c:CHUNK + c + 1],
                            lhsT=xt[:, j, sl],
                            rhs=ones8[:],
                            start=(mm == 1), stop=(mm == n_mm))

            # ---- head squares on DVE (one op each, accum into pack) ----
            sq0 = io.tile([P, 2, COLS], mybir.dt.bfloat16)
            nc.vector.scalar_tensor_tensor(
                out=sq0[:], in0=h0[:], scalar=1.0, in1=h0[:],
                op0=mybir.AluOpType.mult, op1=mybir.AluOpType.mult,
                accum_out=pack[:, 132:133])
            sq1 = io.tile([P, 2, COLS], mybir.dt.bfloat16)
            nc.vector.scalar_tensor_tensor(
                out=sq1[:], in0=h1[:], scalar=1.0, in1=h1[:],
                op0=mybir.AluOpType.mult, op1=mybir.AluOpType.mult,
                accum_out=pack[:, 133:134])

            # ---- tail: one PSUM -> pack copy, one out DMA ----
            nc.vector.tensor_copy(pack[:, 0:BANK_W], bankGC[:])
            nc.sync.dma_start(out_ext[:, :], pack[:])

    nc.finalize()
    return nc


_NC_CACHE = None


def _get_nc():
    global _NC_CACHE
    if _NC_CACHE is None:
        _NC_CACHE = _build()
    return _NC_CACHE


def _finish(packs):
    """Host-side fp64 reduction of the 8 per-core packs."""
    sumsq = 0.0
    colsum_sq = 0.0
    for k in range(N_CORES):
        pk = np.asarray(packs[k], dtype=np.float64)
        g = pk[:, 0:CHUNK]
        sumsq += float(np.trace(g))
        sumsq += float(pk[:, 132:134].sum())
        c = pk[:, CHUNK:CHUNK + N_CHUNKS]          # [q, chunk]
        colsum_sq += float((c * c).sum())
    total = 2.0 * N_CLASSES * sumsq - 2.0 * colsum_sq
    return np.float32(-total / FACTOR)


_LAST_RETRIES = 0


def _run(anchors: np.ndarray, trace: bool = False):
    """Returns (loss_scalar, BassKernelResults)."""
    global _LAST_RETRIES
    x = np.asarray(anchors, dtype=np.float32).reshape(N_CLASSES, D)
    in_maps = [
        {"anchors": np.ascontiguousarray(x[:, i * COLS:(i + 1) * COLS])}
        for i in range(N_CORES)
    ]
    nc = _get_nc()
    # The very first NEFF execution on a freshly-initialized device has
    # been seen to return garbage packs (uninitialized-memory flake).
    # Re-run until the packs pass integrity checks (finite, and the
    # sum-of-squares partials non-negative); steady-state runs are clean.
    _LAST_RETRIES = 0

    def _ok(p):
        if not np.isfinite(p).all():
            return False
        d = np.diagonal(p[:, 0:CHUNK])
        return d.min() > -1e-3 and p[:, 132:134].min() > -1e-3

    for attempt in range(4):
        res = run_bass_kernel_spmd(nc, in_maps,
                                   core_ids=list(range(N_CORES)),
                                   trace=trace)
        packs = [r["out"] for r in res.results]
        if all(_ok(np.asarray(p)) for p in packs):
            break
        _LAST_RETRIES = attempt + 1
    loss = _finish(packs)
    return loss, res


def kernel(anchors: np.ndarray) -> np.ndarray:
    loss, _ = _run(anchors)
    return np.asarray(loss, dtype=np.float32).reshape(())
